# revision 35
# baseline (speedup 1.0000x reference)
"""Complex-valued multi-head attention on 8 Trainium2 NeuronCores.

Sharding: batch(2) x head-pairs(4) -> 8 cores; each core runs one batch
element and 2 heads end-to-end (QKV proj -> complex scores -> |s| softmax
-> AV -> partial W_O), host sums the W_O partials over the 4 cores of each
batch element (tensor-parallel reduce) and transposes to the output layout.

Engine plan (cost-model driven, verifier-safe: at most one PSUM input per
instruction, no GPSIMD access to PSUM, one open PSUM accumulation group
per bank):
- PE: projections emit q/A stacked tensors ([p_r;p_i] x n) and V directly
  in [n, dv] layout; C = [-k_i; k_r] comes from partition-shifted
  negate/copy reads of the A projection PSUM (no extra matmuls).
- Scores stay [k, q]. |s|^2: s_im is extracted+squared by ACT Square for
  k-tiles {2, 10}; the rest are DVE-copied to fp16 and squared on GPSIMD,
  which also does all the adds (SBUF-only). sqrt+exp of the previous
  chunk's bt run in place as 4KB pieces interleaved into the current
  score loop (all sqrt pieces before all exp pieces, so only two
  act-table switches are charged per chunk).
- AV runs at pipeline depth 2 (chunk i-2's probs during chunk i's score
  loop, so exp latency never stalls the PE FIFO), p-stationary per
  (head, 128-q block): one contiguous 16-matmul accumulation into
  [q, dv|rowsum] PSUM using a ones-column padded V, so the softmax
  denominator is column 128 of the same group. Normalization is a
  per-partition tensor_scalar; o is PE-transposed back to [dv, q] for
  W_O. Projection chunks 2..7 are interleaved into qc0's score loop.
"""
import sys

sys.path.insert(0, "/opt/trn_rl_repo")

import numpy as np

B, NQ, NK, R = 2, 2048, 2048, 512
H, DK, DV = 8, 64, 64
NCORES = 8
NCC = 8          # n-chunks for projection streaming (2048/256)
NCW = 256        # projection n-chunk width
QC = 4           # q-chunks in attention (2048/512)
QCW = 512
KT = 16          # k-tiles (2048/128)
VW = 129         # v16 block width (128 dv + ones column)

_CACHE = {}


def _build_nc():
    import concourse.bass as bass
    import concourse.tile as tile
    from concourse import bacc, mybir

    f32 = mybir.dt.float32
    f16 = mybir.dt.float16
    ALU = mybir.AluOpType
    AF = mybir.ActivationFunctionType

    nc = bacc.Bacc("TRN2", target_bir_lowering=False, debug=False,
                   num_devices=NCORES)

    xpack_e = nc.dram_tensor("xpack", [NCC, 128, 24 * NCW], f16,
                             kind="ExternalInput")
    wpack_e = nc.dram_tensor("wpack", [128, 48 * 128], f16,
                             kind="ExternalInput")
    wopack_e = nc.dram_tensor("wopack", [128, 4 * 512], f16,
                              kind="ExternalInput")
    ident_e = nc.dram_tensor("ident", [128, 128], f16, kind="ExternalInput")
    ore_e = nc.dram_tensor("out_re", [512, NQ], f16, kind="ExternalOutput")
    oim_e = nc.dram_tensor("out_im", [512, NQ], f16, kind="ExternalOutput")

    def ecopy(eng, dst, src):
        if eng == "act":
            nc.scalar.copy(dst, src)
        else:
            nc.vector.tensor_copy(dst, src)

    with tile.TileContext(nc) as tc:
      with nc.allow_low_precision(reason="fp16 softmax path"):
        with tc.tile_pool(name="pers", bufs=1) as pers, \
             tc.tile_pool(name="work", bufs=2) as work, \
             tc.tile_pool(name="psA", bufs=1, space="PSUM") as psA:

            # ---- constants ----
            wp = pers.tile([128, 48 * 128], f16, tag="wp")
            wop = pers.tile([128, 4 * 512], f16, tag="wop")
            ident16 = pers.tile([128, 128], f16, tag="ident16")
            eb_exp = pers.tile([128, 1], f32, tag="eb_exp")
            nc.vector.memset(eb_exp[:], -1.5)          # exp(mag - 1.5)

            q_sb = [pers.tile([128, NQ], f16, tag=f"q_sb{h}",
                              name=f"q_sb{h}") for h in (0, 1)]
            A_sb = [pers.tile([128, NK], f16, tag=f"A_sb{h}",
                              name=f"A_sb{h}") for h in (0, 1)]
            C_sb = [pers.tile([128, NK], f16, tag=f"C_sb{h}",
                              name=f"C_sb{h}") for h in (0, 1)]
            # V in [n, dv] blocks of width 129; col 128 of each block = 1.0
            v16_h = [pers.tile([128, KT * VW], f16, tag=f"v16_h{h}",
                               name=f"v16_h{h}") for h in (0, 1)]
            nc.gpsimd.memset(v16_h[0][:], 1.0)
            nc.gpsimd.memset(v16_h[1][:], 1.0)
            oT_h = [pers.tile([128, NQ], f16, tag=f"oT_h{h}",
                              name=f"oT_h{h}") for h in (0, 1)]

            # weight/const loads: first q-spec weights, then xt0 piece 0
            # (issued inside emit_proj(0)), then the rest
            nc.sync.dma_start(wp[:, 0:2048], wpack_e[:, 0:2048])

            def wblk(si, p):
                return wp[:, (si * 8 + p) * 128:(si * 8 + p + 1) * 128]

            # layout-A specs: (dest, tensor-pair, psum tag, copy engine)
            specsA = [(q_sb[0], 0, "s_re", "act"), (q_sb[1], 0, "s_im", "dve"),
                      (A_sb[0], 1, "s_re", "act"), (A_sb[1], 1, "s_im", "dve")]

            def emit_proj(ncc):
                if ncc == 0:
                    xt = work.tile([128, 24 * NCW], f16, tag="xt")
                    nc.sync.dma_start(xt[:, 0:2048], xpack_e[0][:, 0:2048])
                    nc.sync.dma_start(wp[:, 2048:6144], wpack_e[:, 2048:6144])
                    nc.sync.dma_start(xt[:, 2048:4096],
                                      xpack_e[0][:, 2048:4096])
                    nc.sync.dma_start(xt[:, 4096:6144],
                                      xpack_e[0][:, 4096:6144])
                    nc.sync.dma_start(wop[:], wopack_e[:])
                    nc.sync.dma_start(ident16[:], ident_e[:])
                else:
                    xt = work.tile([128, 24 * NCW], f16, tag="xt")
                    nc.sync.dma_start(xt[:], xpack_e[ncc])
                cs = slice(ncc * NCW, (ncc + 1) * NCW)

                def xblk(slot, c0, cw):
                    return xt[:, slot * NCW + c0:slot * NCW + c0 + cw]

                for si, (dest, tp, ptag, ceng) in enumerate(specsA):
                    pj = psA.tile([128, 1024], f32, tag=ptag,
                                  name=f"pj_{ncc}_{si}")
                    for rc in range(4):
                        nc.tensor.matmul(pj[:, 0:NCW], wblk(si, rc),
                                         xblk(2 * tp * 4 + rc, 0, NCW),
                                         start=(rc == 0), stop=False)
                    for rc in range(4):
                        nc.tensor.matmul(pj[:, 0:NCW], wblk(si, 4 + rc),
                                         xblk((2 * tp + 1) * 4 + rc, 0, NCW),
                                         start=False, stop=(rc == 3))
                    ecopy(ceng, dest[:, cs], pj[:, 0:NCW])
                    if si >= 2:
                        # C = [-k_i; k_r] via partition-shifted psum reads
                        h = si - 2
                        e2 = "act" if h == 0 else "dve"
                        if h == 0:
                            nc.scalar.mul(C_sb[h][0:64, cs],
                                          pj[64:128, 0:NCW], -1.0)
                        else:
                            nc.vector.tensor_scalar(
                                C_sb[h][0:64, cs], pj[64:128, 0:NCW],
                                -1.0, None, ALU.mult)
                        ecopy(e2, C_sb[h][64:128, cs], pj[0:64, 0:NCW])

                # layout-B V projection: [n, dv] blocks
                for h in (0, 1):
                    si = 4 + h
                    for nb in (0, 1):
                        vps = psA.tile([128, VW], f32, tag="oacc", bufs=3,
                                       name=f"vps_{ncc}_{h}_{nb}")
                        for rc in range(4):
                            nc.tensor.matmul(vps[:, 0:128],
                                             xblk(16 + rc, nb * 128, 128),
                                             wblk(si, rc),
                                             start=(rc == 0), stop=False)
                        for rc in range(4):
                            nc.tensor.matmul(vps[:, 0:128],
                                             xblk(20 + rc, nb * 128, 128),
                                             wblk(si, 4 + rc),
                                             start=False, stop=(rc == 3))
                        nt = 2 * ncc + nb
                        ecopy("act" if h == 0 else "dve",
                              v16_h[h][:, nt * VW:nt * VW + 128],
                              vps[:, 0:128])

            emit_proj(0)
            emit_proj(1)

            # ---- attention ----
            def emit_av_group(pend, g):
                h, qb = g // 4, g % 4
                pbt = pend["bt"]
                oacc = psA.tile([128, VW], f32, tag="oacc", bufs=3,
                                name=f"oacc_{pend['qc']}_{g}")
                pend["oacc"][g] = oacc
                for kt in range(KT):
                    stat = pbt[:, kt * 1024 + h * 512 + qb * 128:
                               kt * 1024 + h * 512 + qb * 128 + 128]
                    nc.tensor.matmul(oacc[:], stat,
                                     v16_h[h][:, kt * VW:(kt + 1) * VW],
                                     start=(kt == 0), stop=(kt == KT - 1))

            def emit_post(pend, g):
                h, qb = g // 4, g % 4
                oacc, pqs, pqc = pend["oacc"][g], pend["qs"], pend["qc"]
                recip = work.tile([128, 1], f32, tag="recip", bufs=2,
                                  name=f"recip_{pqc}_{g}")
                nc.vector.reciprocal(recip[:], oacc[:, 128:129])
                o_sb = work.tile([128, 128], f16, tag="osb", bufs=2,
                                 name=f"osb_{pqc}_{g}")
                nc.vector.tensor_scalar(o_sb[:], oacc[:, 0:128],
                                        recip[:], None, ALU.mult)
                otp = psA.tile([128, 256], f16, tag="otps",
                               name=f"otp_{pqc}_{g}")
                dst = otp[:, (g % 2) * 128:(g % 2) * 128 + 128]
                nc.tensor.transpose(dst, o_sb[:], ident16[:])
                nc.vector.tensor_copy(
                    oT_h[h][:, pqs.start + qb * 128:
                            pqs.start + qb * 128 + 128], dst)

            def emit_wo(pend):
                pqs, pqc = pend["qs"], pend["qc"]

                def wob(hh, out, Rc):
                    base = (hh * 2 + out) * 512 + Rc * 128
                    return wop[:, base:base + 128]

                for half_rc in (0, 1):
                    wo_re = psA.tile([128, 1024], f32, tag="s_re",
                                     name=f"wore_{pqc}_{half_rc}")
                    wo_im = psA.tile([128, 1024], f32, tag="s_im",
                                     name=f"woim_{pqc}_{half_rc}")
                    for i in (0, 1):
                        Rc = half_rc * 2 + i
                        cs = slice(i * 512, (i + 1) * 512)
                        nc.tensor.matmul(wo_re[:, cs], wob(0, 0, Rc),
                                         oT_h[0][:, pqs],
                                         start=True, stop=False)
                        nc.tensor.matmul(wo_re[:, cs], wob(1, 0, Rc),
                                         oT_h[1][:, pqs],
                                         start=False, stop=True)
                        nc.tensor.matmul(wo_im[:, cs], wob(0, 1, Rc),
                                         oT_h[0][:, pqs],
                                         start=True, stop=False)
                        nc.tensor.matmul(wo_im[:, cs], wob(1, 1, Rc),
                                         oT_h[1][:, pqs],
                                         start=False, stop=True)
                    st_re = work.tile([128, 1024], f16, tag="st_re",
                                      name=f"stre_{pqc}_{half_rc}")
                    nc.vector.tensor_copy(st_re[:], wo_re[:])
                    st_im = work.tile([128, 1024], f16, tag="st_im",
                                      name=f"stim_{pqc}_{half_rc}")
                    nc.vector.tensor_copy(st_im[:], wo_im[:])
                    for i in (0, 1):
                        Rc = half_rc * 2 + i
                        cs = slice(i * 512, (i + 1) * 512)
                        nc.sync.dma_start(
                            ore_e[Rc * 128:(Rc + 1) * 128, pqs], st_re[:, cs])
                        nc.sync.dma_start(
                            oim_e[Rc * 128:(Rc + 1) * 128, pqs], st_im[:, cs])

            ACT_IM = {0, 4, 8, 12}  # im-square on ACT for these k-tiles

            def emit_tr_piece(tbt, piece, npc=8):
                # sqrt/exp of an older chunk's bt in 2*npc pieces:
                # pieces 0..npc-1 sqrt, npc..2*npc-1 exp
                pw = KT * 1024 // npc
                if piece < npc:
                    sl = slice(piece * pw, (piece + 1) * pw)
                    nc.scalar.activation(tbt[:, sl], tbt[:, sl], AF.Sqrt,
                                         scale=1.0 / 64.0)
                else:
                    sl = slice((piece - npc) * pw, (piece - npc + 1) * pw)
                    nc.scalar.activation(tbt[:, sl], tbt[:, sl], AF.Exp,
                                         bias=eb_exp[:])

            pend_q = []
            tr_pend = None
            for qc in range(QC):
                qs = slice(qc * QCW, (qc + 1) * QCW)
                bt = work.tile([128, KT * 1024], f16, tag="bt", bufs=3,
                               name=f"bt_{qc}")
                # process the 2-chunks-old pending (its exp is long done)
                pend = pend_q.pop(0) if len(pend_q) == 2 else None
                for kt in range(KT):
                    ks = slice(kt * 128, (kt + 1) * 128)
                    s_re = psA.tile([128, 1024], f32, tag="s_re",
                                    name=f"s_re_{qc}_{kt}")
                    s_im = psA.tile([128, 1024], f32, tag="s_im",
                                    name=f"s_im_{qc}_{kt}")
                    for h in (0, 1):
                        col = slice(h * 512, h * 512 + 512)
                        nc.tensor.matmul(s_re[:, col], A_sb[h][:, ks],
                                         q_sb[h][:, qs],
                                         start=True, stop=True)
                        nc.tensor.matmul(s_im[:, col], C_sb[h][:, ks],
                                         q_sb[h][:, qs],
                                         start=True, stop=True)
                    if qc == 0 and kt % 2 == 1 and kt <= 11:
                        emit_proj(2 + (kt - 1) // 2)
                    if pend is not None and kt % 2 == 1:
                        emit_av_group(pend, kt // 2)
                        if kt >= 5:
                            emit_post(pend, (kt - 5) // 2)
                    # |s|^2 extraction
                    t_re = work.tile([128, 1024], f16, tag="tre", bufs=2,
                                     name=f"tre_{qc}_{kt}")
                    nc.vector.tensor_copy(t_re[:], s_re[:])
                    sq_re = work.tile([128, 1024], f16, tag="sqre", bufs=2,
                                      name=f"sqre_{qc}_{kt}")
                    nc.gpsimd.tensor_tensor(sq_re[:], t_re[:], t_re[:],
                                            ALU.mult)
                    if kt in ACT_IM:
                        sq_im = work.tile([128, 1024], f16, tag="sqim",
                                          bufs=2, name=f"sqim_{qc}_{kt}")
                        nc.scalar.square(sq_im[:], s_im[:])
                    else:
                        t_im = work.tile([128, 1024], f16, tag="tim", bufs=2,
                                         name=f"tim_{qc}_{kt}")
                        nc.vector.tensor_copy(t_im[:], s_im[:])
                        sq_im = work.tile([128, 1024], f16, tag="sqim",
                                          bufs=2, name=f"sqim_{qc}_{kt}")
                        nc.gpsimd.tensor_tensor(sq_im[:], t_im[:], t_im[:],
                                                ALU.mult)
                    nc.gpsimd.tensor_tensor(bt[:, kt * 1024:(kt + 1) * 1024],
                                            sq_re[:], sq_im[:], ALU.add)
                    if tr_pend is not None:
                        emit_tr_piece(tr_pend, kt, npc=8)
                if pend is not None:
                    emit_post(pend, 6)
                    emit_post(pend, 7)
                    emit_wo(pend)
                pend_q.append({"bt": bt, "qs": qs, "qc": qc,
                               "oacc": [None] * 8})
                tr_pend = bt

            # tail: transcendentals for the last chunk, then flush AV of the
            # last two chunks (qc2's exp finished during qc3's window)
            for piece in range(8):
                emit_tr_piece(tr_pend, piece, npc=4)
            for pend in pend_q:
                for g in range(8):
                    emit_av_group(pend, g)
                    if g >= 2:
                        emit_post(pend, g - 2)
                emit_post(pend, 6)
                emit_post(pend, 7)
                emit_wo(pend)

    nc.finalize()
    return nc


def _get_nc():
    if "nc" not in _CACHE:
        _CACHE["nc"] = _build_nc()
    return _CACHE["nc"]


def _core_inputs(c, inputs):
    b = c // 4
    h0 = 2 * (c % 4)

    # xpack[ncc, p, s*NCW + f] = xT[rc*128 + p, ncc*NCW + f], s = t*4 + rc
    xpack = np.empty((NCC, 128, 24, NCW), np.float16)
    for t, name in enumerate(
            ("Q_real", "Q_imag", "K_real", "K_imag", "V_real", "V_imag")):
        xT = np.ascontiguousarray(inputs[name][b].T)          # (512, 2048)
        blk = xT.reshape(4, 128, NCC, NCW)                    # (rc, p, ncc, f)
        xpack[:, :, t * 4:(t + 1) * 4, :] = blk.transpose(2, 1, 0, 3)
    xpack = xpack.reshape(NCC, 128, 24 * NCW)

    # wpack: 6 specs (q_h0, q_h1, A_h0, A_h1, v_h0, v_h1) x 8 pass-blocks
    blocks = []
    for nr, ni in (("wq_r", "wq_i"), ("wk_r", "wk_i"), ("wv_r", "wv_i")):
        for hh in (0, 1):
            rows = slice((h0 + hh) * 64, (h0 + hh) * 64 + 64)
            wr = inputs[nr][rows].astype(np.float32)
            wi = inputs[ni][rows].astype(np.float32)
            w1 = np.vstack([wr, wi]).T       # (512, 128) for x_re passes
            w2 = np.vstack([-wi, wr]).T      # (512, 128) for x_im passes
            for rc in range(4):
                blocks.append(w1[rc * 128:(rc + 1) * 128])
            for rc in range(4):
                blocks.append(w2[rc * 128:(rc + 1) * 128])
    wpack = np.concatenate(blocks, axis=1).astype(np.float16)

    # wopack: per (hh, out) one [128, 512] stationary strip
    wo_blocks = []
    for hh in (0, 1):
        hsl = slice((h0 + hh) * 64, (h0 + hh) * 64 + 64)
        wr = inputs["wo_r"][:, hsl].astype(np.float32)        # (512, 64)
        wi = inputs["wo_i"][:, hsl].astype(np.float32)
        wo_blocks.append(np.vstack([wr.T, -wi.T]))            # out_re
        wo_blocks.append(np.vstack([wi.T, wr.T]))             # out_im
    wopack = np.concatenate(wo_blocks, axis=1).astype(np.float16)

    return {
        "xpack": np.ascontiguousarray(xpack),
        "wpack": np.ascontiguousarray(wpack),
        "wopack": np.ascontiguousarray(wopack),
        "ident": np.eye(128, dtype=np.float16),
    }


def kernel(**inputs):
    from concourse.bass_utils import run_bass_kernel_spmd

    nc = _get_nc()
    in_maps = [_core_inputs(c, inputs) for c in range(NCORES)]
    res = run_bass_kernel_spmd(nc, in_maps, list(range(NCORES)))
    out = np.empty((B, NQ, R, 2), np.float32)
    for b in range(B):
        re = np.zeros((512, NQ), np.float32)
        im = np.zeros((512, NQ), np.float32)
        for c in range(b * 4, b * 4 + 4):
            re += res.results[c]["out_re"].astype(np.float32)
            im += res.results[c]["out_im"].astype(np.float32)
        out[b, :, :, 0] = re.T
        out[b, :, :, 1] = im.T
    return out


# revision 37
# speedup vs baseline: 1.0005x; 1.0005x over previous
"""Complex-valued multi-head attention on 8 Trainium2 NeuronCores.

Sharding: batch(2) x head-pairs(4) -> 8 cores; each core runs one batch
element and 2 heads end-to-end (QKV proj -> complex scores -> |s| softmax
-> AV -> partial W_O), host sums the W_O partials over the 4 cores of each
batch element (tensor-parallel reduce) and transposes to the output layout.

Engine plan (cost-model driven, verifier-safe: at most one PSUM input per
instruction, no GPSIMD access to PSUM, one open PSUM accumulation group
per bank):
- PE: projections emit q/A stacked tensors ([p_r;p_i] x n) and V directly
  in [n, dv] layout; C = [-k_i; k_r] comes from partition-shifted
  negate/copy reads of the A projection PSUM (no extra matmuls).
- Scores stay [k, q]. |s|^2: s_im is extracted+squared by ACT Square for
  k-tiles {2, 10}; the rest are DVE-copied to fp16 and squared on GPSIMD,
  which also does all the adds (SBUF-only). sqrt+exp of the previous
  chunk's bt run in place as 4KB pieces interleaved into the current
  score loop (all sqrt pieces before all exp pieces, so only two
  act-table switches are charged per chunk).
- AV runs at pipeline depth 2 (chunk i-2's probs during chunk i's score
  loop, so exp latency never stalls the PE FIFO), p-stationary per
  (head, 128-q block): one contiguous 16-matmul accumulation into
  [q, dv|rowsum] PSUM using a ones-column padded V, so the softmax
  denominator is column 128 of the same group. Normalization is a
  per-partition tensor_scalar; o is PE-transposed back to [dv, q] for
  W_O. Projection chunks 2..7 are interleaved into qc0's score loop.
"""
import sys

sys.path.insert(0, "/opt/trn_rl_repo")

import numpy as np

B, NQ, NK, R = 2, 2048, 2048, 512
H, DK, DV = 8, 64, 64
NCORES = 8
NCC = 8          # n-chunks for projection streaming (2048/256)
NCW = 256        # projection n-chunk width
QC = 4           # q-chunks in attention (2048/512)
QCW = 512
KT = 16          # k-tiles (2048/128)
VW = 129         # v16 block width (128 dv + ones column)

_CACHE = {}


def _build_nc():
    import concourse.bass as bass
    import concourse.tile as tile
    from concourse import bacc, mybir

    f32 = mybir.dt.float32
    f16 = mybir.dt.float16
    ALU = mybir.AluOpType
    AF = mybir.ActivationFunctionType

    nc = bacc.Bacc("TRN2", target_bir_lowering=False, debug=False,
                   num_devices=NCORES)

    xpack_e = nc.dram_tensor("xpack", [NCC, 128, 24 * NCW], f16,
                             kind="ExternalInput")
    wpack_e = nc.dram_tensor("wpack", [128, 48 * 128], f16,
                             kind="ExternalInput")
    wopack_e = nc.dram_tensor("wopack", [128, 4 * 512], f16,
                              kind="ExternalInput")
    ident_e = nc.dram_tensor("ident", [128, 128], f16, kind="ExternalInput")
    ore_e = nc.dram_tensor("out_re", [512, NQ], f16, kind="ExternalOutput")
    oim_e = nc.dram_tensor("out_im", [512, NQ], f16, kind="ExternalOutput")

    def ecopy(eng, dst, src):
        if eng == "act":
            nc.scalar.copy(dst, src)
        else:
            nc.vector.tensor_copy(dst, src)

    with tile.TileContext(nc) as tc:
      with nc.allow_low_precision(reason="fp16 softmax path"):
        with tc.tile_pool(name="pers", bufs=1) as pers, \
             tc.tile_pool(name="work", bufs=2) as work, \
             tc.tile_pool(name="psA", bufs=1, space="PSUM") as psA:

            # ---- constants ----
            wp = pers.tile([128, 48 * 128], f16, tag="wp")
            wop = pers.tile([128, 4 * 512], f16, tag="wop")
            ident16 = pers.tile([128, 128], f16, tag="ident16")
            eb_exp = pers.tile([128, 1], f32, tag="eb_exp")
            nc.vector.memset(eb_exp[:], -1.5)          # exp(mag - 1.5)

            q_sb = [pers.tile([128, NQ], f16, tag=f"q_sb{h}",
                              name=f"q_sb{h}") for h in (0, 1)]
            A_sb = [pers.tile([128, NK], f16, tag=f"A_sb{h}",
                              name=f"A_sb{h}") for h in (0, 1)]
            C_sb = [pers.tile([128, NK], f16, tag=f"C_sb{h}",
                              name=f"C_sb{h}") for h in (0, 1)]
            # V in [n, dv] blocks of width 129; col 128 of each block = 1.0
            v16_h = [pers.tile([128, KT * VW], f16, tag=f"v16_h{h}",
                               name=f"v16_h{h}") for h in (0, 1)]
            nc.gpsimd.memset(v16_h[0][:], 1.0)
            nc.gpsimd.memset(v16_h[1][:], 1.0)
            oT_h = [pers.tile([128, NQ], f16, tag=f"oT_h{h}",
                              name=f"oT_h{h}") for h in (0, 1)]

            # weight/const loads: first q-spec weights, then xt0 piece 0
            # (issued inside emit_proj(0)), then the rest
            nc.sync.dma_start(wp[:, 0:2048], wpack_e[:, 0:2048])

            def wblk(si, p):
                return wp[:, (si * 8 + p) * 128:(si * 8 + p + 1) * 128]

            # layout-A specs: (dest, tensor-pair, psum tag, copy engine)
            specsA = [(q_sb[0], 0, "s_re", "act"), (q_sb[1], 0, "s_im", "dve"),
                      (A_sb[0], 1, "s_re", "act"), (A_sb[1], 1, "s_im", "dve")]

            def emit_proj(ncc):
                if ncc == 0:
                    xt = work.tile([128, 24 * NCW], f16, tag="xt")
                    nc.sync.dma_start(xt[:, 0:2048], xpack_e[0][:, 0:2048])
                    nc.sync.dma_start(wp[:, 2048:6144], wpack_e[:, 2048:6144])
                    nc.sync.dma_start(xt[:, 2048:4096],
                                      xpack_e[0][:, 2048:4096])
                    nc.sync.dma_start(xt[:, 4096:6144],
                                      xpack_e[0][:, 4096:6144])
                    nc.sync.dma_start(wop[:], wopack_e[:])
                    nc.sync.dma_start(ident16[:], ident_e[:])
                else:
                    xt = work.tile([128, 24 * NCW], f16, tag="xt")
                    nc.sync.dma_start(xt[:], xpack_e[ncc])
                cs = slice(ncc * NCW, (ncc + 1) * NCW)

                def xblk(slot, c0, cw):
                    return xt[:, slot * NCW + c0:slot * NCW + c0 + cw]

                for si, (dest, tp, ptag, ceng) in enumerate(specsA):
                    pj = psA.tile([128, 1024], f32, tag=ptag,
                                  name=f"pj_{ncc}_{si}")
                    for rc in range(4):
                        nc.tensor.matmul(pj[:, 0:NCW], wblk(si, rc),
                                         xblk(2 * tp * 4 + rc, 0, NCW),
                                         start=(rc == 0), stop=False)
                    for rc in range(4):
                        nc.tensor.matmul(pj[:, 0:NCW], wblk(si, 4 + rc),
                                         xblk((2 * tp + 1) * 4 + rc, 0, NCW),
                                         start=False, stop=(rc == 3))
                    ecopy(ceng, dest[:, cs], pj[:, 0:NCW])
                    if si >= 2:
                        # C = [-k_i; k_r] via partition-shifted psum reads
                        h = si - 2
                        e2 = "act" if h == 0 else "dve"
                        if h == 0:
                            nc.scalar.mul(C_sb[h][0:64, cs],
                                          pj[64:128, 0:NCW], -1.0)
                        else:
                            nc.vector.tensor_scalar(
                                C_sb[h][0:64, cs], pj[64:128, 0:NCW],
                                -1.0, None, ALU.mult)
                        ecopy(e2, C_sb[h][64:128, cs], pj[0:64, 0:NCW])

                # layout-B V projection: [n, dv] blocks
                for h in (0, 1):
                    si = 4 + h
                    for nb in (0, 1):
                        vps = psA.tile([128, VW], f32, tag="oacc", bufs=3,
                                       name=f"vps_{ncc}_{h}_{nb}")
                        for rc in range(4):
                            nc.tensor.matmul(vps[:, 0:128],
                                             xblk(16 + rc, nb * 128, 128),
                                             wblk(si, rc),
                                             start=(rc == 0), stop=False)
                        for rc in range(4):
                            nc.tensor.matmul(vps[:, 0:128],
                                             xblk(20 + rc, nb * 128, 128),
                                             wblk(si, 4 + rc),
                                             start=False, stop=(rc == 3))
                        nt = 2 * ncc + nb
                        ecopy("act" if h == 0 else "dve",
                              v16_h[h][:, nt * VW:nt * VW + 128],
                              vps[:, 0:128])

            emit_proj(0)
            emit_proj(1)

            # ---- attention ----
            def emit_av_group(pend, g):
                h, qb = g // 4, g % 4
                pbt = pend["bt"]
                oacc = psA.tile([128, VW], f32, tag="oacc", bufs=3,
                                name=f"oacc_{pend['qc']}_{g}")
                pend["oacc"][g] = oacc
                for kt in range(KT):
                    stat = pbt[:, kt * 1024 + h * 512 + qb * 128:
                               kt * 1024 + h * 512 + qb * 128 + 128]
                    nc.tensor.matmul(oacc[:], stat,
                                     v16_h[h][:, kt * VW:(kt + 1) * VW],
                                     start=(kt == 0), stop=(kt == KT - 1))

            def emit_post(pend, g):
                h, qb = g // 4, g % 4
                oacc, pqs, pqc = pend["oacc"][g], pend["qs"], pend["qc"]
                recip = work.tile([128, 1], f32, tag="recip", bufs=2,
                                  name=f"recip_{pqc}_{g}")
                nc.vector.reciprocal(recip[:], oacc[:, 128:129])
                o_sb = work.tile([128, 128], f16, tag="osb", bufs=2,
                                 name=f"osb_{pqc}_{g}")
                nc.vector.tensor_scalar(o_sb[:], oacc[:, 0:128],
                                        recip[:], None, ALU.mult)
                otp = psA.tile([128, 256], f16, tag="otps",
                               name=f"otp_{pqc}_{g}")
                dst = otp[:, (g % 2) * 128:(g % 2) * 128 + 128]
                nc.tensor.transpose(dst, o_sb[:], ident16[:])
                nc.vector.tensor_copy(
                    oT_h[h][:, pqs.start + qb * 128:
                            pqs.start + qb * 128 + 128], dst)

            def emit_wo(pend):
                pqs, pqc = pend["qs"], pend["qc"]

                def wob(hh, out, Rc):
                    base = (hh * 2 + out) * 512 + Rc * 128
                    return wop[:, base:base + 128]

                for half_rc in (0, 1):
                    wo_re = psA.tile([128, 1024], f32, tag="s_re",
                                     name=f"wore_{pqc}_{half_rc}")
                    wo_im = psA.tile([128, 1024], f32, tag="s_im",
                                     name=f"woim_{pqc}_{half_rc}")
                    for i in (0, 1):
                        Rc = half_rc * 2 + i
                        cs = slice(i * 512, (i + 1) * 512)
                        nc.tensor.matmul(wo_re[:, cs], wob(0, 0, Rc),
                                         oT_h[0][:, pqs],
                                         start=True, stop=False)
                        nc.tensor.matmul(wo_re[:, cs], wob(1, 0, Rc),
                                         oT_h[1][:, pqs],
                                         start=False, stop=True)
                        nc.tensor.matmul(wo_im[:, cs], wob(0, 1, Rc),
                                         oT_h[0][:, pqs],
                                         start=True, stop=False)
                        nc.tensor.matmul(wo_im[:, cs], wob(1, 1, Rc),
                                         oT_h[1][:, pqs],
                                         start=False, stop=True)
                    st_re = work.tile([128, 1024], f16, tag="st_re",
                                      name=f"stre_{pqc}_{half_rc}")
                    nc.vector.tensor_copy(st_re[:], wo_re[:])
                    st_im = work.tile([128, 1024], f16, tag="st_im",
                                      name=f"stim_{pqc}_{half_rc}")
                    nc.vector.tensor_copy(st_im[:], wo_im[:])
                    for i in (0, 1):
                        Rc = half_rc * 2 + i
                        cs = slice(i * 512, (i + 1) * 512)
                        nc.sync.dma_start(
                            ore_e[Rc * 128:(Rc + 1) * 128, pqs], st_re[:, cs])
                        nc.sync.dma_start(
                            oim_e[Rc * 128:(Rc + 1) * 128, pqs], st_im[:, cs])

            ACT_IM = {0, 4, 8, 12}  # im-square on ACT for these k-tiles

            def emit_tr_piece(tbt, piece, npc=8):
                # sqrt/exp of an older chunk's bt in 2*npc pieces:
                # pieces 0..npc-1 sqrt, npc..2*npc-1 exp
                pw = KT * 1024 // npc
                if piece < npc:
                    sl = slice(piece * pw, (piece + 1) * pw)
                    nc.scalar.activation(tbt[:, sl], tbt[:, sl], AF.Sqrt,
                                         scale=1.0 / 64.0)
                else:
                    sl = slice((piece - npc) * pw, (piece - npc + 1) * pw)
                    nc.scalar.activation(tbt[:, sl], tbt[:, sl], AF.Exp,
                                         bias=eb_exp[:])

            pend_q = []
            tr_pend = None
            for qc in range(QC):
                qs = slice(qc * QCW, (qc + 1) * QCW)
                bt = work.tile([128, KT * 1024], f16, tag="bt", bufs=3,
                               name=f"bt_{qc}")
                # process the 2-chunks-old pending (its exp is long done)
                pend = pend_q.pop(0) if len(pend_q) == 2 else None
                for kt in range(KT):
                    ks = slice(kt * 128, (kt + 1) * 128)
                    s_re = psA.tile([128, 1024], f32, tag="s_re",
                                    name=f"s_re_{qc}_{kt}")
                    s_im = psA.tile([128, 1024], f32, tag="s_im",
                                    name=f"s_im_{qc}_{kt}")
                    for h in (0, 1):
                        col = slice(h * 512, h * 512 + 512)
                        nc.tensor.matmul(s_re[:, col], A_sb[h][:, ks],
                                         q_sb[h][:, qs],
                                         start=True, stop=True)
                        nc.tensor.matmul(s_im[:, col], C_sb[h][:, ks],
                                         q_sb[h][:, qs],
                                         start=True, stop=True)
                    if qc == 0 and kt % 2 == 1 and kt <= 11:
                        emit_proj(2 + (kt - 1) // 2)
                    if pend is not None and kt % 2 == 1:
                        emit_av_group(pend, kt // 2)
                        if kt >= 5:
                            emit_post(pend, (kt - 5) // 2)
                    # |s|^2 extraction
                    t_re = work.tile([128, 1024], f16, tag="tre", bufs=2,
                                     name=f"tre_{qc}_{kt}")
                    nc.vector.tensor_copy(t_re[:], s_re[:])
                    sq_re = work.tile([128, 1024], f16, tag="sqre", bufs=2,
                                      name=f"sqre_{qc}_{kt}")
                    nc.gpsimd.tensor_tensor(sq_re[:], t_re[:], t_re[:],
                                            ALU.mult)
                    if kt in ACT_IM:
                        sq_im = work.tile([128, 1024], f16, tag="sqim",
                                          bufs=2, name=f"sqim_{qc}_{kt}")
                        nc.scalar.square(sq_im[:], s_im[:])
                    else:
                        t_im = work.tile([128, 1024], f16, tag="tim", bufs=2,
                                         name=f"tim_{qc}_{kt}")
                        nc.vector.tensor_copy(t_im[:], s_im[:])
                        sq_im = work.tile([128, 1024], f16, tag="sqim",
                                          bufs=2, name=f"sqim_{qc}_{kt}")
                        nc.gpsimd.tensor_tensor(sq_im[:], t_im[:], t_im[:],
                                                ALU.mult)
                    nc.gpsimd.tensor_tensor(bt[:, kt * 1024:(kt + 1) * 1024],
                                            sq_re[:], sq_im[:], ALU.add)
                    if tr_pend is not None:
                        emit_tr_piece(tr_pend, kt, npc=8)
                if pend is not None:
                    emit_post(pend, 6)
                    emit_post(pend, 7)
                    emit_wo(pend)
                pend_q.append({"bt": bt, "qs": qs, "qc": qc,
                               "oacc": [None] * 8})
                tr_pend = bt

            # tail: transcendentals for the last chunk, then flush AV of the
            # last two chunks (qc2's exp finished during qc3's window)
            for piece in range(8):
                emit_tr_piece(tr_pend, piece, npc=4)
            for pend in pend_q:
                for g in range(8):
                    emit_av_group(pend, g)
                    if g >= 2:
                        emit_post(pend, g - 2)
                emit_post(pend, 6)
                emit_post(pend, 7)
                emit_wo(pend)

    nc.finalize()
    return nc


def _get_nc():
    if "nc" not in _CACHE:
        _CACHE["nc"] = _build_nc()
    return _CACHE["nc"]


def _core_inputs(c, inputs):
    b = c // 4
    h0 = 2 * (c % 4)

    # xpack[ncc, p, s*NCW + f] = xT[rc*128 + p, ncc*NCW + f], s = t*4 + rc
    xpack = np.empty((NCC, 128, 24, NCW), np.float16)
    for t, name in enumerate(
            ("Q_real", "Q_imag", "K_real", "K_imag", "V_real", "V_imag")):
        xT = np.ascontiguousarray(inputs[name][b].T)          # (512, 2048)
        blk = xT.reshape(4, 128, NCC, NCW)                    # (rc, p, ncc, f)
        xpack[:, :, t * 4:(t + 1) * 4, :] = blk.transpose(2, 1, 0, 3)
    xpack = xpack.reshape(NCC, 128, 24 * NCW)

    # wpack: 6 specs (q_h0, q_h1, A_h0, A_h1, v_h0, v_h1) x 8 pass-blocks
    blocks = []
    for nr, ni in (("wq_r", "wq_i"), ("wk_r", "wk_i"), ("wv_r", "wv_i")):
        for hh in (0, 1):
            rows = slice((h0 + hh) * 64, (h0 + hh) * 64 + 64)
            wr = inputs[nr][rows].astype(np.float32)
            wi = inputs[ni][rows].astype(np.float32)
            w1 = np.vstack([wr, wi]).T       # (512, 128) for x_re passes
            w2 = np.vstack([-wi, wr]).T      # (512, 128) for x_im passes
            for rc in range(4):
                blocks.append(w1[rc * 128:(rc + 1) * 128])
            for rc in range(4):
                blocks.append(w2[rc * 128:(rc + 1) * 128])
    wpack = np.concatenate(blocks, axis=1).astype(np.float16)

    # wopack: per (hh, out) one [128, 512] stationary strip
    wo_blocks = []
    for hh in (0, 1):
        hsl = slice((h0 + hh) * 64, (h0 + hh) * 64 + 64)
        wr = inputs["wo_r"][:, hsl].astype(np.float32)        # (512, 64)
        wi = inputs["wo_i"][:, hsl].astype(np.float32)
        wo_blocks.append(np.vstack([wr.T, -wi.T]))            # out_re
        wo_blocks.append(np.vstack([wi.T, wr.T]))             # out_im
    wopack = np.concatenate(wo_blocks, axis=1).astype(np.float16)

    return {
        "xpack": np.ascontiguousarray(xpack),
        "wpack": np.ascontiguousarray(wpack),
        "wopack": np.ascontiguousarray(wopack),
        "ident": np.eye(128, dtype=np.float16),
    }


def kernel(**inputs):
    from concourse.bass_utils import run_bass_kernel_spmd

    nc = _get_nc()
    in_maps = [_core_inputs(c, inputs) for c in range(NCORES)]
    res = run_bass_kernel_spmd(nc, in_maps, list(range(NCORES)))
    out = np.empty((B, NQ, R, 2), np.float32)
    for b in range(B):
        re = np.zeros((512, NQ), np.float32)
        im = np.zeros((512, NQ), np.float32)
        for c in range(b * 4, b * 4 + 4):
            re += res.results[c]["out_re"].astype(np.float32)
            im += res.results[c]["out_im"].astype(np.float32)
        out[b, :, :, 0] = re.T
        out[b, :, :, 1] = im.T
    return out
